# revision 16
# baseline (speedup 1.0000x reference)
"""DSQG attention kernel for 8 Trainium2 NeuronCores — v2 (transposed layout).

Sharding: core c = (b, half): batch b = c//2, heads [half*8, half*8+8).

All on-chip tensors live in TRANSPOSED layout [channels on partitions, seq on
free]: causal lag shifts become free-dim slices (no partition-alignment issues,
no DMA window fetches). Per core:

  ph1: qT/kT/vT/gT [512ch, 4096] = W^T-stationary matmuls over xT (bias via
       ones-row). kT/vT carry 512 zero-pad columns for causal lookback.
  A:   per (ptile t, lag L): P = qT_t * kT_t[:, n-L]  (DVE, bf16 2x)
       scores via selector-matmul  sc[32s+h, n] += E_t^T @ P  (TensorE reduce
       over d; 24 lags in 6 PSUM "quad" banks, 32-aligned slots).
  B:   esc = exp(SCALE*sc + ln wl)  (ScalarE, dup-offset pos_bias weights wl
       folded via per-partition bias); Z via selector-matmul; rz = 1/Z.
  C:   abc = S_{t,s}^T @ esc  (TensorE row-broadcast) -> ScalarE copy to bf16
       -> prod = abc * vT_t[:, n-L]  (DVE 2x) -> out_t += I^T @ prod
       (TensorE identity-matmul accumulation in PSUM).
  m:   mT_t = sigmoid(gT_t) * (R4_t^T @ rz) * out_t   (normalize at the end)
  ph3: yT[Do, n] = wo^T-stationary matmuls over mT; host transposes, sums the
       two half partials and adds b_out.
"""
import sys

sys.path.insert(0, "/opt/trn_rl_repo")

import numpy as np
import ml_dtypes

N_SCALES = 11
N_TAPS = 4
OFFSETS = [(1 << j) * tau for j in range(N_SCALES) for tau in range(N_TAPS)]
LAGS = sorted(set(OFFSETS))  # 24 distinct lags
NL = len(LAGS)
NQ = NL // 4  # 6 quad groups of 4 lags
B, N, D, H = 4, 4096, 1024, 16
HD = 64
HH = 8           # heads per core
CW = HH * HD     # 512 channels per core
NT = 4           # channel ptiles per core (128 ch each, 2 heads per ptile)
KE = 1024        # contraction rows (biases folded into psum copies)
KP = KE // 128   # 8
PADN = 512       # zero columns in front of kT/vT for causal lookback
NTile = 512      # seq columns per ph2 tile
NNT = N // NTile  # 8
SCALE = HD ** -0.5

_CACHE = {}

# const packing offsets within cb [128, CB_W] bf16
CB_E4 = 0                  # 4 x [128, 32] selectors E_t
CB_E2 = CB_E4 + 4 * 32     # [128, 8] Z selector
CB_S2 = CB_E2 + 8          # 16 x [128, 128] broadcast selectors S_{t,s}
CB_I = CB_S2 + 16 * 128    # [128, 128] identity
CB_W = CB_I + 128


def _live(L, n0):
    # window [n0-L, n0-L+NTile) intersects valid v/k rows (reads stay inside
    # the PADN zero region otherwise)
    return L < n0 + NTile


def _build_program():
    import concourse.bacc as bacc
    import concourse.mybir as mybir
    import concourse.tile as tile

    bf16 = mybir.dt.bfloat16
    f32 = mybir.dt.float32
    Act = mybir.ActivationFunctionType

    nc = bacc.Bacc("TRN2", target_bir_lowering=False, debug=False, num_devices=8)
    xT = nc.dram_tensor("xT", [KE, N], bf16, kind="ExternalInput").ap()
    wqg = nc.dram_tensor("wqg", [KE, 4 * CW], bf16, kind="ExternalInput").ap()
    wo = nc.dram_tensor("wo", [CW, D], bf16, kind="ExternalInput").ap()
    cb = nc.dram_tensor("cb", [128, CB_W], bf16, kind="ExternalInput").ap()
    cf = nc.dram_tensor("cf", [128, NQ], f32, kind="ExternalInput").ap()
    r4 = nc.dram_tensor("r4", [HH, 4 * 128], f32, kind="ExternalInput").ap()
    bv = nc.dram_tensor("bv", [128, 16], f32, kind="ExternalInput").ap()
    yT = nc.dram_tensor("yT", [D, N], f32, kind="ExternalOutput").ap()

    with tile.TileContext(nc) as tc:
        with (
            tc.tile_pool(name="constp", bufs=1) as constp,
            tc.tile_pool(name="pers", bufs=1) as pers,
        ):
            cbt = constp.tile([128, CB_W], bf16, tag="cb")
            nc.sync.dma_start(out=cbt[:], in_=cb[:, :])
            cft = constp.tile([128, NQ], f32, tag="cf")
            nc.sync.dma_start(out=cft[:], in_=cf[:, :])
            r4t = constp.tile([HH, 4 * 128], f32, tag="r4")
            nc.sync.dma_start(out=r4t[:], in_=r4[:, :])
            bvt = constp.tile([128, 16], f32, tag="bv")
            nc.sync.dma_start(out=bvt[:], in_=bv[:, :])

            qT = [pers.tile([128, N], bf16, tag=f"qT{t}", name=f"qT{t}") for t in range(NT)]
            kT = [pers.tile([128, PADN + N], bf16, tag=f"kT{t}", name=f"kT{t}") for t in range(NT)]
            vT = [pers.tile([128, PADN + N], bf16, tag=f"vT{t}", name=f"vT{t}") for t in range(NT)]
            gT = [pers.tile([128, N], bf16, tag=f"gT{t}", name=f"gT{t}") for t in range(NT)]
            mT = qT  # qT columns are dead after stage A reads them; reuse as m
            for t in range(NT):
                nc.vector.memset(kT[t][:, 0:PADN], 0.0)
                nc.vector.memset(vT[t][:, 0:PADN], 0.0)

            # ---- Phase 1: transposed qkv+gate projection ----
            with (
                tc.tile_pool(name="xp", bufs=1) as xp,
                tc.tile_pool(name="wp", bufs=4) as wp,
                tc.tile_pool(name="pp1", bufs=8, space="PSUM") as pp1,
            ):
                for half in range(2):
                    xts = []
                    for kp in range(KP):
                        xt = xp.tile([128, N // 2], bf16, tag=f"x{kp}")
                        nc.sync.dma_start(
                            out=xt[:],
                            in_=xT[kp * 128:(kp + 1) * 128,
                                   half * (N // 2):(half + 1) * (N // 2)])
                        xts.append(xt)
                    for ch in (0, 1, 2, 3, 4, 5, 6, 7, 8, 9, 10, 11, 12, 13, 14, 15) if half == 1 else (4, 5, 6, 7, 0, 1, 2, 3, 8, 9, 10, 11, 12, 13, 14, 15):
                        ps = [pp1.tile([128, NTile], f32, tag="p1", name="p1") for _ in range(4)]
                        for kp in range(KP):
                            wt = wp.tile([128, 128], bf16, tag="wt")
                            eng = nc.sync if (kp % 2 == 0) else nc.scalar
                            eng.dma_start(
                                out=wt[:],
                                in_=wqg[kp * 128:(kp + 1) * 128,
                                        ch * 128:(ch + 1) * 128])
                            for ns in range(4):
                                nc.tensor.matmul(
                                    out=ps[ns][:],
                                    lhsT=wt[:],
                                    rhs=xts[kp][:, ns * NTile:(ns + 1) * NTile],
                                    start=(kp == 0), stop=(kp == KP - 1))
                        grp, t = divmod(ch, NT)
                        dst = [qT, kT, vT, gT][grp][t]
                        off = (PADN if grp in (1, 2) else 0) + half * (N // 2)
                        fn = Act.Sigmoid if grp == 3 else Act.Identity
                        for ns in range(4):
                            nc.scalar.activation(
                                dst[:, off + ns * NTile: off + (ns + 1) * NTile],
                                ps[ns][:], fn, bias=bvt[:, ch:ch + 1])

            # ---- Phase 2: attention middle ----
            with (
                tc.tile_pool(name="work", bufs=7) as work,
                tc.tile_pool(name="pwork", bufs=26) as pwork,
                tc.tile_pool(name="escp", bufs=8) as escp,
                tc.tile_pool(name="rzp", bufs=2) as rzp,
                tc.tile_pool(name="scp", bufs=2, space="PSUM") as scp,
                tc.tile_pool(name="zp", bufs=1, space="PSUM") as zp,
                tc.tile_pool(name="abcp", bufs=3, space="PSUM") as abcp,
                tc.tile_pool(name="outp", bufs=2, space="PSUM") as outp,
            ):
                def emit_pmul(nt, q, s, t, ptiles, ahead=False):
                    n0 = nt * NTile
                    L = LAGS[q * 4 + s]
                    pt = pwork.tile([128, NTile], bf16, tag="P", name="P")
                    nc.vector.tensor_mul(
                        pt[:],
                        qT[t][:, n0:n0 + NTile],
                        kT[t][:, PADN + n0 - L: PADN + n0 - L + NTile])
                    ptiles[(q, s, t)] = pt

                def amul_list(nt):
                    n0 = nt * NTile
                    return [(q, s, t) for q in range(NQ) for t in range(NT)
                            for s in range(4) if _live(LAGS[q * 4 + s], n0)]

                pm_ahead = {}  # P tiles pre-produced during previous C loop
                for (qq, ss, tt) in amul_list(0)[:20]:
                    emit_pmul(0, qq, ss, tt, pm_ahead, ahead=True)

                for nt in range(NNT):
                    n0 = nt * NTile
                    escs = []
                    zt = zp.tile([HH, NTile], f32, tag="z")
                    for q in range(NQ):
                        sct = scp.tile([128, NTile], f32, tag="sc")
                        slots = []
                        for s in range(4):
                            L = LAGS[q * 4 + s]
                            if not _live(L, n0):
                                nc.vector.memset(sct[32 * s:32 * s + 32, :], 0.0)
                            else:
                                slots.append(s)
                        for t in range(NT):  # t-outer: E_t stationary reused
                            for s in slots:
                                if (q, s, t) not in pm_ahead:
                                    emit_pmul(nt, q, s, t, pm_ahead)
                                pt = pm_ahead.pop((q, s, t))
                                nc.tensor.matmul(
                                    out=sct[32 * s:32 * s + 32, :],
                                    lhsT=cbt[:, CB_E4 + 32 * t: CB_E4 + 32 * (t + 1)],
                                    rhs=pt[:],
                                    start=(t == 0), stop=(t == NT - 1),
                                    tile_position=(0, 32 * s))
                        esc = escp.tile([128, NTile], bf16, tag="esc")
                        nc.scalar.activation(esc[:], sct[:], Act.Exp,
                                             scale=SCALE, bias=cft[:, q:q + 1])
                        escs.append(esc)
                        nc.tensor.matmul(
                            out=zt[:],
                            lhsT=cbt[:, CB_E2: CB_E2 + HH],
                            rhs=esc[:],
                            start=(q == 0), stop=(q == NQ - 1))
                    rz = rzp.tile([HH, NTile], f32, tag="rz")
                    nc.vector.reciprocal_approx_fast(rz[:], zt[:])
                    # A-muls of nt+1 to interleave into this nt's C loop
                    next_am = amul_list(nt + 1) if nt + 1 < NNT else []
                    next_am = list(reversed(next_am))  # pop() from the front

                    live = [(q, s) for q in range(NQ) for s in range(4)
                            if _live(LAGS[q * 4 + s], n0)]
                    for t in range(NT):
                        outps = outp.tile([128, NTile], f32, tag="out")
                        nlive = len(live)
                        nids = nlive
                        state = {"ids": 0}
                        pend = []  # skew software pipeline: abc runs ahead

                        def flush_pair(outps=outps, pend=pend, state=state,
                                       nids=nids):
                            rhs = pend.pop(0)
                            i = state["ids"]
                            nc.tensor.matmul(
                                out=outps[:],
                                lhsT=cbt[:, CB_I: CB_I + 128],
                                rhs=rhs[:],
                                start=(i == 0), stop=(i == nids - 1))
                            state["ids"] = i + 1

                        for s in range(4):  # s-runs: S2_{t,s} stationary reused
                            for q in range(NQ):
                                if not _live(LAGS[q * 4 + s], n0):
                                    continue
                                if next_am and len(pm_ahead) < 23:
                                    qq, ss, tt = next_am.pop()
                                    emit_pmul(nt + 1, qq, ss, tt, pm_ahead,
                                              ahead=True)
                                L = LAGS[q * 4 + s]
                                abc = abcp.tile([128, NTile], f32, tag="abc")
                                nc.tensor.matmul(
                                    out=abc[:],
                                    lhsT=cbt[:, CB_S2 + (t * 4 + s) * 128:
                                             CB_S2 + (t * 4 + s + 1) * 128],
                                    rhs=escs[q][:],
                                    start=True, stop=True)
                                absb = work.tile([128, NTile], bf16, tag="absb")
                                nc.scalar.activation(absb[:], abc[:], Act.Copy)
                                prod = work.tile([128, NTile], bf16, tag="prod")
                                nc.vector.tensor_mul(
                                    prod[:], absb[:],
                                    vT[t][:, PADN + n0 - L: PADN + n0 - L + NTile])
                                pend.append(prod)
                            while len(pend) > 1:  # drain at s-run boundary
                                flush_pair()
                        while pend:
                            flush_pair()
                        # finalize: m = sigmoid(g) * (1/Z bcast) * gathered
                        rzb = abcp.tile([128, NTile], f32, tag="abc")
                        nc.tensor.matmul(
                            out=rzb[:],
                            lhsT=r4t[:, t * 128:(t + 1) * 128],
                            rhs=rz[:],
                            start=True, stop=True)
                        tmp = work.tile([128, NTile], bf16, tag="tmp")
                        nc.vector.tensor_mul(tmp[:], gT[t][:, n0:n0 + NTile],
                                             rzb[:])
                        nc.vector.tensor_mul(
                            mT[t][:, n0:n0 + NTile], tmp[:], outps[:])

            # ---- Phase 3: transposed out projection ----
            with (
                tc.tile_pool(name="wp3", bufs=4) as wp3,
                tc.tile_pool(name="ys", bufs=4) as ys,
                tc.tile_pool(name="pp3", bufs=8, space="PSUM") as pp3,
            ):
                for do in range(D // 128):
                    pss = [pp3.tile([128, NTile], f32, tag="p3", name="p3") for _ in range(NNT)]
                    for ct in range(NT):
                        wt3 = wp3.tile([128, 128], bf16, tag="wt3")
                        nc.sync.dma_start(
                            out=wt3[:],
                            in_=wo[ct * 128:(ct + 1) * 128, do * 128:(do + 1) * 128])
                        for ns in range(NNT):
                            nc.tensor.matmul(
                                out=pss[ns][:],
                                lhsT=wt3[:],
                                rhs=mT[ct][:, ns * NTile:(ns + 1) * NTile],
                                start=(ct == 0), stop=(ct == NT - 1))
                    for ns in range(NNT):
                        yst = ys.tile([128, NTile], f32, tag="yst")
                        nc.scalar.activation(yst[:], pss[ns][:], Act.Copy)
                        nc.sync.dma_start(
                            out=yT[do * 128:(do + 1) * 128,
                                   ns * NTile:(ns + 1) * NTile],
                            in_=yst[:])

    nc.compile()
    return nc


def _get_program():
    if "nc" not in _CACHE:
        _CACHE["nc"] = _build_program()
    return _CACHE["nc"]


def _core_inputs(x, w_qkv, b_qkv, w_gate, b_gate, w_out, pos_bias, b, half):
    bf = ml_dtypes.bfloat16
    cs = slice(half * CW, (half + 1) * CW)

    xTa = x[b].T.astype(bf)

    wqg = np.empty((KE, 4 * CW), dtype=np.float32)
    wqg[:, 0:CW] = w_qkv[:, cs]
    wqg[:, CW:2 * CW] = w_qkv[:, D + cs.start:D + cs.stop]
    wqg[:, 2 * CW:3 * CW] = w_qkv[:, 2 * D + cs.start:2 * D + cs.stop]
    wqg[:, 3 * CW:4 * CW] = w_gate[:, cs]

    bcat = np.concatenate([b_qkv[cs], b_qkv[D + cs.start:D + cs.stop],
                           b_qkv[2 * D + cs.start:2 * D + cs.stop],
                           b_gate[cs]]).astype(np.float32)
    bva = bcat.reshape(16, 128).T.copy()  # bva[p, ch] = bias[ch*128 + p]

    # wl[h, j] = sum over duplicate offsets of exp(pos_bias[i, h])  (local heads)
    wl = np.zeros((HH, NL), dtype=np.float64)
    for i, off in enumerate(OFFSETS):
        j = LAGS.index(off)
        wl[:, j] += np.exp(pos_bias[i, half * HH:(half + 1) * HH].astype(np.float64))
    lnwl = np.log(wl)  # [HH, NL]

    # bf16 consts: selectors
    cba = np.zeros((128, CB_W), dtype=bf)
    p = np.arange(128)
    for t in range(NT):
        for j in range(2):  # local heads 2t, 2t+1
            col = CB_E4 + 32 * t + 2 * t + j
            cba[:, col] = (p // 64 == j).astype(bf)
    for h in range(HH):
        cba[:, CB_E2 + h] = (p % 32 == h).astype(bf)
    for t in range(NT):
        for s in range(4):
            blk = np.zeros((128, 128), dtype=bf)
            for pd in range(128):
                blk[32 * s + 2 * t + pd // 64, pd] = 1
            cba[:, CB_S2 + (t * 4 + s) * 128: CB_S2 + (t * 4 + s + 1) * 128] = blk
    cba[:, CB_I: CB_I + 128] = np.eye(128, dtype=np.float32).astype(bf)

    # f32 consts: lnwl at rows 32*s + h, col q
    cfa = np.zeros((128, NQ), dtype=np.float32)
    for q in range(NQ):
        for s in range(4):
            for h in range(HH):
                cfa[32 * s + h, q] = lnwl[h, q * 4 + s]

    r4a = np.zeros((HH, 4 * 128), dtype=np.float32)
    for t in range(NT):
        for pd in range(128):
            r4a[2 * t + pd // 64, t * 128 + pd] = 1.0

    return {
        "xT": xTa,
        "wqg": wqg.astype(bf),
        "wo": w_out[cs, :].astype(bf),
        "cb": cba,
        "cf": cfa,
        "r4": r4a,
        "bv": bva,
    }


def _in_maps(inputs):
    return [
        _core_inputs(inputs["x"], inputs["w_qkv"], inputs["b_qkv"], inputs["w_gate"],
                     inputs["b_gate"], inputs["w_out"], inputs["pos_bias"], c // 2, c % 2)
        for c in range(8)
    ]


def kernel(x, w_qkv, b_qkv, w_out, b_out, w_gate, b_gate, pos_bias):
    from concourse.bass_utils import run_bass_kernel_spmd

    x = np.asarray(x, dtype=np.float32)
    w_qkv = np.asarray(w_qkv, dtype=np.float32)
    b_qkv = np.asarray(b_qkv, dtype=np.float32)
    w_out = np.asarray(w_out, dtype=np.float32)
    b_out = np.asarray(b_out, dtype=np.float32)
    w_gate = np.asarray(w_gate, dtype=np.float32)
    b_gate = np.asarray(b_gate, dtype=np.float32)
    pos_bias = np.asarray(pos_bias, dtype=np.float32)

    nc = _get_program()
    in_maps = _in_maps({
        "x": x, "w_qkv": w_qkv, "b_qkv": b_qkv, "w_gate": w_gate,
        "b_gate": b_gate, "w_out": w_out, "pos_bias": pos_bias,
    })
    res = run_bass_kernel_spmd(nc, in_maps, core_ids=list(range(8)))
    out = np.empty((B, N, D), dtype=np.float32)
    for b in range(B):
        out[b] = (res.results[2 * b]["yT"] + res.results[2 * b + 1]["yT"]).T \
            + b_out[None, :]
    return out


# revision 17
# speedup vs baseline: 1.2219x; 1.2219x over previous
"""DSQG attention kernel for 8 Trainium2 NeuronCores — v2 (transposed layout).

Sharding: core c = (b, half): batch b = c//2, heads [half*8, half*8+8).

All on-chip tensors live in TRANSPOSED layout [channels on partitions, seq on
free]: causal lag shifts become free-dim slices (no partition-alignment issues,
no DMA window fetches). Per core:

  ph1: qT/kT/vT/gT [512ch, 4096] = W^T-stationary matmuls over xT (bias via
       ones-row). kT/vT carry 512 zero-pad columns for causal lookback.
  A:   per (ptile t, lag L): P = qT_t * kT_t[:, n-L]  (DVE, bf16 2x)
       scores via selector-matmul  sc[32s+h, n] += E_t^T @ P  (TensorE reduce
       over d; 24 lags in 6 PSUM "quad" banks, 32-aligned slots).
  B:   esc = exp(SCALE*sc + ln wl)  (ScalarE, dup-offset pos_bias weights wl
       folded via per-partition bias); Z via selector-matmul; rz = 1/Z.
  C:   abc = S_{t,s}^T @ esc  (TensorE row-broadcast) -> ScalarE copy to bf16
       -> prod = abc * vT_t[:, n-L]  (DVE 2x) -> out_t += I^T @ prod
       (TensorE identity-matmul accumulation in PSUM).
  m:   mT_t = sigmoid(gT_t) * (R4_t^T @ rz) * out_t   (normalize at the end)
  ph3: yT[Do, n] = wo^T-stationary matmuls over mT; host transposes, sums the
       two half partials and adds b_out.
"""
import sys

sys.path.insert(0, "/opt/trn_rl_repo")

import numpy as np
import ml_dtypes

N_SCALES = 11
N_TAPS = 4
OFFSETS = [(1 << j) * tau for j in range(N_SCALES) for tau in range(N_TAPS)]
LAGS = sorted(set(OFFSETS))  # 24 distinct lags
NL = len(LAGS)
NQ = NL // 4  # 6 quad groups of 4 lags
B, N, D, H = 4, 4096, 1024, 16
HD = 64
HH = 8           # heads per core
CW = HH * HD     # 512 channels per core
NT = 4           # channel ptiles per core (128 ch each, 2 heads per ptile)
KE = 1024        # contraction rows (biases folded into psum copies)
KP = KE // 128   # 8
PADN = 512       # zero columns in front of kT/vT for causal lookback
NTile = 512      # seq columns per ph2 tile
NNT = N // NTile  # 8
SCALE = HD ** -0.5

_CACHE = {}

# const packing offsets within cb [128, CB_W] bf16
CB_E4 = 0                  # 4 x [128, 32] selectors E_t
CB_E2 = CB_E4 + 4 * 32     # [128, 8] Z selector
CB_S2 = CB_E2 + 8          # 16 x [128, 128] broadcast selectors S_{t,s}
CB_I = CB_S2 + 16 * 128    # [128, 128] identity
CB_W = CB_I + 128


def _live(L, n0):
    # window [n0-L, n0-L+NTile) intersects valid v/k rows (reads stay inside
    # the PADN zero region otherwise)
    return L < n0 + NTile


def _build_program():
    import concourse.bacc as bacc
    import concourse.mybir as mybir
    import concourse.tile as tile

    bf16 = mybir.dt.bfloat16
    f32 = mybir.dt.float32
    Act = mybir.ActivationFunctionType

    nc = bacc.Bacc("TRN2", target_bir_lowering=False, debug=False, num_devices=8)
    xT = nc.dram_tensor("xT", [KE, N], bf16, kind="ExternalInput").ap()
    wqg = nc.dram_tensor("wqg", [KE, 4 * CW], bf16, kind="ExternalInput").ap()
    wo = nc.dram_tensor("wo", [CW, D], bf16, kind="ExternalInput").ap()
    cb = nc.dram_tensor("cb", [128, CB_W], bf16, kind="ExternalInput").ap()
    cf = nc.dram_tensor("cf", [128, NQ], f32, kind="ExternalInput").ap()
    r4 = nc.dram_tensor("r4", [HH, 4 * 128], f32, kind="ExternalInput").ap()
    bv = nc.dram_tensor("bv", [128, 16], f32, kind="ExternalInput").ap()
    yT = nc.dram_tensor("yT", [D, N], f32, kind="ExternalOutput").ap()

    with tile.TileContext(nc) as tc:
        with (
            tc.tile_pool(name="constp", bufs=1) as constp,
            tc.tile_pool(name="pers", bufs=1) as pers,
        ):
            cbt = constp.tile([128, CB_W], bf16, tag="cb")
            nc.sync.dma_start(out=cbt[:], in_=cb[:, :])
            cft = constp.tile([128, NQ], f32, tag="cf")
            nc.sync.dma_start(out=cft[:], in_=cf[:, :])
            r4t = constp.tile([HH, 4 * 128], f32, tag="r4")
            nc.sync.dma_start(out=r4t[:], in_=r4[:, :])
            bvt = constp.tile([128, 16], f32, tag="bv")
            nc.sync.dma_start(out=bvt[:], in_=bv[:, :])

            qT = [pers.tile([128, N], bf16, tag=f"qT{t}", name=f"qT{t}") for t in range(NT)]
            kT = [pers.tile([128, PADN + N], bf16, tag=f"kT{t}", name=f"kT{t}") for t in range(NT)]
            vT = [pers.tile([128, PADN + N], bf16, tag=f"vT{t}", name=f"vT{t}") for t in range(NT)]
            gT = [pers.tile([128, N], bf16, tag=f"gT{t}", name=f"gT{t}") for t in range(NT)]
            mT = qT  # qT columns are dead after stage A reads them; reuse as m
            for t in range(NT):
                nc.vector.memset(kT[t][:, 0:PADN], 0.0)
                nc.vector.memset(vT[t][:, 0:PADN], 0.0)

            # ---- Phase 1: transposed qkv+gate projection ----
            with (
                tc.tile_pool(name="xp", bufs=1) as xp,
                tc.tile_pool(name="wp", bufs=4) as wp,
                tc.tile_pool(name="pp1", bufs=8, space="PSUM") as pp1,
            ):
                for half in range(2):
                    xts = []
                    for kp in range(KP):
                        xt = xp.tile([128, N // 2], bf16, tag=f"x{kp}")
                        nc.sync.dma_start(
                            out=xt[:],
                            in_=xT[kp * 128:(kp + 1) * 128,
                                   half * (N // 2):(half + 1) * (N // 2)])
                        xts.append(xt)
                    for ch in (0, 1, 2, 3, 4, 5, 6, 7, 8, 9, 10, 11, 12, 13, 14, 15) if half == 1 else (4, 5, 6, 7, 0, 1, 2, 3, 8, 9, 10, 11, 12, 13, 14, 15):
                        ps = [pp1.tile([128, NTile], f32, tag="p1", name="p1") for _ in range(4)]
                        for kp in range(KP):
                            wt = wp.tile([128, 128], bf16, tag="wt")
                            eng = nc.sync if (kp % 2 == 0) else nc.scalar
                            eng.dma_start(
                                out=wt[:],
                                in_=wqg[kp * 128:(kp + 1) * 128,
                                        ch * 128:(ch + 1) * 128])
                            for ns in range(4):
                                nc.tensor.matmul(
                                    out=ps[ns][:],
                                    lhsT=wt[:],
                                    rhs=xts[kp][:, ns * NTile:(ns + 1) * NTile],
                                    start=(kp == 0), stop=(kp == KP - 1))
                        grp, t = divmod(ch, NT)
                        dst = [qT, kT, vT, gT][grp][t]
                        off = (PADN if grp in (1, 2) else 0) + half * (N // 2)
                        fn = Act.Sigmoid if grp == 3 else Act.Identity
                        for ns in range(4):
                            nc.scalar.activation(
                                dst[:, off + ns * NTile: off + (ns + 1) * NTile],
                                ps[ns][:], fn, bias=bvt[:, ch:ch + 1])

            # ---- Phase 2: attention middle ----
            with (
                tc.tile_pool(name="work", bufs=4) as work,
                tc.tile_pool(name="pwork", bufs=26) as pwork,
                tc.tile_pool(name="escp", bufs=8) as escp,
                tc.tile_pool(name="rzp", bufs=2) as rzp,
                tc.tile_pool(name="scp", bufs=2, space="PSUM") as scp,
                tc.tile_pool(name="zp", bufs=1, space="PSUM") as zp,
                tc.tile_pool(name="abcp", bufs=3, space="PSUM") as abcp,
                tc.tile_pool(name="outp", bufs=2, space="PSUM") as outp,
            ):
                def emit_pmul(nt, q, s, t, ptiles, ahead=False):
                    n0 = nt * NTile
                    L = LAGS[q * 4 + s]
                    pt = pwork.tile([128, NTile], bf16, tag="P", name="P")
                    nc.vector.tensor_mul(
                        pt[:],
                        qT[t][:, n0:n0 + NTile],
                        kT[t][:, PADN + n0 - L: PADN + n0 - L + NTile])
                    ptiles[(q, s, t)] = pt

                def amul_list(nt):
                    n0 = nt * NTile
                    return [(q, s, t) for q in range(NQ) for t in range(NT)
                            for s in range(4) if _live(LAGS[q * 4 + s], n0)]

                pm_ahead = {}  # P tiles pre-produced during previous C loop
                for (qq, ss, tt) in amul_list(0)[:20]:
                    emit_pmul(0, qq, ss, tt, pm_ahead, ahead=True)

                for nt in range(NNT):
                    n0 = nt * NTile
                    escs = []
                    zt = zp.tile([HH, NTile], f32, tag="z")
                    for q in range(NQ):
                        sct = scp.tile([128, NTile], f32, tag="sc")
                        slots = []
                        for s in range(4):
                            L = LAGS[q * 4 + s]
                            if not _live(L, n0):
                                nc.vector.memset(sct[32 * s:32 * s + 32, :], 0.0)
                            else:
                                slots.append(s)
                        for t in range(NT):  # t-outer: E_t stationary reused
                            for s in slots:
                                if (q, s, t) not in pm_ahead:
                                    emit_pmul(nt, q, s, t, pm_ahead)
                                pt = pm_ahead.pop((q, s, t))
                                nc.tensor.matmul(
                                    out=sct[32 * s:32 * s + 32, :],
                                    lhsT=cbt[:, CB_E4 + 32 * t: CB_E4 + 32 * (t + 1)],
                                    rhs=pt[:],
                                    start=(t == 0), stop=(t == NT - 1),
                                    tile_position=(0, 32 * s))
                        esc = escp.tile([128, NTile], bf16, tag="esc")
                        nc.scalar.activation(esc[:], sct[:], Act.Exp,
                                             scale=SCALE, bias=cft[:, q:q + 1])
                        escs.append(esc)
                        nc.tensor.matmul(
                            out=zt[:],
                            lhsT=cbt[:, CB_E2: CB_E2 + HH],
                            rhs=esc[:],
                            start=(q == 0), stop=(q == NQ - 1))
                    rz = rzp.tile([HH, NTile], f32, tag="rz")
                    nc.vector.reciprocal_approx_fast(rz[:], zt[:])
                    # A-muls of nt+1 to interleave into this nt's C loop
                    next_am = amul_list(nt + 1) if nt + 1 < NNT else []
                    next_am = list(reversed(next_am))  # pop() from the front

                    live = [(q, s) for q in range(NQ) for s in range(4)
                            if _live(LAGS[q * 4 + s], n0)]
                    for t in range(NT):
                        outps = outp.tile([128, NTile], f32, tag="out")
                        nlive = len(live)
                        nids = nlive
                        state = {"ids": 0}
                        pend = []  # skew software pipeline: abc runs ahead

                        def flush_pair(outps=outps, pend=pend, state=state,
                                       nids=nids):
                            rhs = pend.pop(0)
                            i = state["ids"]
                            nc.tensor.matmul(
                                out=outps[:],
                                lhsT=cbt[:, CB_I: CB_I + 128],
                                rhs=rhs[:],
                                start=(i == 0), stop=(i == nids - 1))
                            state["ids"] = i + 1

                        for s in range(4):  # s-runs: S2_{t,s} stationary reused
                            for q in range(NQ):
                                if not _live(LAGS[q * 4 + s], n0):
                                    continue
                                if next_am and len(pm_ahead) < 23:
                                    qq, ss, tt = next_am.pop()
                                    emit_pmul(nt + 1, qq, ss, tt, pm_ahead,
                                              ahead=True)
                                L = LAGS[q * 4 + s]
                                abc = abcp.tile([128, NTile], f32, tag="abc")
                                nc.tensor.matmul(
                                    out=abc[:],
                                    lhsT=cbt[:, CB_S2 + (t * 4 + s) * 128:
                                             CB_S2 + (t * 4 + s + 1) * 128],
                                    rhs=escs[q][:],
                                    start=True, stop=True)
                                absb = work.tile([128, NTile], bf16, tag="absb")
                                nc.scalar.activation(absb[:], abc[:], Act.Copy)
                                prod = work.tile([128, NTile], bf16, tag="prod")
                                nc.vector.tensor_mul(
                                    prod[:], absb[:],
                                    vT[t][:, PADN + n0 - L: PADN + n0 - L + NTile])
                                pend.append(prod)
                                if len(pend) >= 3:
                                    flush_pair()
                        while pend:
                            flush_pair()
                        # finalize: m = sigmoid(g) * (1/Z bcast) * gathered
                        rzb = abcp.tile([128, NTile], f32, tag="abc")
                        nc.tensor.matmul(
                            out=rzb[:],
                            lhsT=r4t[:, t * 128:(t + 1) * 128],
                            rhs=rz[:],
                            start=True, stop=True)
                        tmp = work.tile([128, NTile], bf16, tag="tmp")
                        nc.vector.tensor_mul(tmp[:], gT[t][:, n0:n0 + NTile],
                                             rzb[:])
                        nc.vector.tensor_mul(
                            mT[t][:, n0:n0 + NTile], tmp[:], outps[:])

            # ---- Phase 3: transposed out projection ----
            with (
                tc.tile_pool(name="wp3", bufs=4) as wp3,
                tc.tile_pool(name="ys", bufs=4) as ys,
                tc.tile_pool(name="pp3", bufs=8, space="PSUM") as pp3,
            ):
                for do in range(D // 128):
                    pss = [pp3.tile([128, NTile], f32, tag="p3", name="p3") for _ in range(NNT)]
                    for ct in range(NT):
                        wt3 = wp3.tile([128, 128], bf16, tag="wt3")
                        nc.sync.dma_start(
                            out=wt3[:],
                            in_=wo[ct * 128:(ct + 1) * 128, do * 128:(do + 1) * 128])
                        for ns in range(NNT):
                            nc.tensor.matmul(
                                out=pss[ns][:],
                                lhsT=wt3[:],
                                rhs=mT[ct][:, ns * NTile:(ns + 1) * NTile],
                                start=(ct == 0), stop=(ct == NT - 1))
                    for ns in range(NNT):
                        yst = ys.tile([128, NTile], f32, tag="yst")
                        nc.scalar.activation(yst[:], pss[ns][:], Act.Copy)
                        nc.sync.dma_start(
                            out=yT[do * 128:(do + 1) * 128,
                                   ns * NTile:(ns + 1) * NTile],
                            in_=yst[:])

    nc.compile()
    return nc


def _get_program():
    if "nc" not in _CACHE:
        _CACHE["nc"] = _build_program()
    return _CACHE["nc"]


def _core_inputs(x, w_qkv, b_qkv, w_gate, b_gate, w_out, pos_bias, b, half):
    bf = ml_dtypes.bfloat16
    cs = slice(half * CW, (half + 1) * CW)

    xTa = x[b].T.astype(bf)

    wqg = np.empty((KE, 4 * CW), dtype=np.float32)
    wqg[:, 0:CW] = w_qkv[:, cs]
    wqg[:, CW:2 * CW] = w_qkv[:, D + cs.start:D + cs.stop]
    wqg[:, 2 * CW:3 * CW] = w_qkv[:, 2 * D + cs.start:2 * D + cs.stop]
    wqg[:, 3 * CW:4 * CW] = w_gate[:, cs]

    bcat = np.concatenate([b_qkv[cs], b_qkv[D + cs.start:D + cs.stop],
                           b_qkv[2 * D + cs.start:2 * D + cs.stop],
                           b_gate[cs]]).astype(np.float32)
    bva = bcat.reshape(16, 128).T.copy()  # bva[p, ch] = bias[ch*128 + p]

    # wl[h, j] = sum over duplicate offsets of exp(pos_bias[i, h])  (local heads)
    wl = np.zeros((HH, NL), dtype=np.float64)
    for i, off in enumerate(OFFSETS):
        j = LAGS.index(off)
        wl[:, j] += np.exp(pos_bias[i, half * HH:(half + 1) * HH].astype(np.float64))
    lnwl = np.log(wl)  # [HH, NL]

    # bf16 consts: selectors
    cba = np.zeros((128, CB_W), dtype=bf)
    p = np.arange(128)
    for t in range(NT):
        for j in range(2):  # local heads 2t, 2t+1
            col = CB_E4 + 32 * t + 2 * t + j
            cba[:, col] = (p // 64 == j).astype(bf)
    for h in range(HH):
        cba[:, CB_E2 + h] = (p % 32 == h).astype(bf)
    for t in range(NT):
        for s in range(4):
            blk = np.zeros((128, 128), dtype=bf)
            for pd in range(128):
                blk[32 * s + 2 * t + pd // 64, pd] = 1
            cba[:, CB_S2 + (t * 4 + s) * 128: CB_S2 + (t * 4 + s + 1) * 128] = blk
    cba[:, CB_I: CB_I + 128] = np.eye(128, dtype=np.float32).astype(bf)

    # f32 consts: lnwl at rows 32*s + h, col q
    cfa = np.zeros((128, NQ), dtype=np.float32)
    for q in range(NQ):
        for s in range(4):
            for h in range(HH):
                cfa[32 * s + h, q] = lnwl[h, q * 4 + s]

    r4a = np.zeros((HH, 4 * 128), dtype=np.float32)
    for t in range(NT):
        for pd in range(128):
            r4a[2 * t + pd // 64, t * 128 + pd] = 1.0

    return {
        "xT": xTa,
        "wqg": wqg.astype(bf),
        "wo": w_out[cs, :].astype(bf),
        "cb": cba,
        "cf": cfa,
        "r4": r4a,
        "bv": bva,
    }


def _in_maps(inputs):
    return [
        _core_inputs(inputs["x"], inputs["w_qkv"], inputs["b_qkv"], inputs["w_gate"],
                     inputs["b_gate"], inputs["w_out"], inputs["pos_bias"], c // 2, c % 2)
        for c in range(8)
    ]


def kernel(x, w_qkv, b_qkv, w_out, b_out, w_gate, b_gate, pos_bias):
    from concourse.bass_utils import run_bass_kernel_spmd

    x = np.asarray(x, dtype=np.float32)
    w_qkv = np.asarray(w_qkv, dtype=np.float32)
    b_qkv = np.asarray(b_qkv, dtype=np.float32)
    w_out = np.asarray(w_out, dtype=np.float32)
    b_out = np.asarray(b_out, dtype=np.float32)
    w_gate = np.asarray(w_gate, dtype=np.float32)
    b_gate = np.asarray(b_gate, dtype=np.float32)
    pos_bias = np.asarray(pos_bias, dtype=np.float32)

    nc = _get_program()
    in_maps = _in_maps({
        "x": x, "w_qkv": w_qkv, "b_qkv": b_qkv, "w_gate": w_gate,
        "b_gate": b_gate, "w_out": w_out, "pos_bias": pos_bias,
    })
    res = run_bass_kernel_spmd(nc, in_maps, core_ids=list(range(8)))
    out = np.empty((B, N, D), dtype=np.float32)
    for b in range(B):
        out[b] = (res.results[2 * b]["yT"] + res.results[2 * b + 1]["yT"]).T \
            + b_out[None, :]
    return out
